# revision 47
# baseline (speedup 1.0000x reference)
"""Trainium2 Bass kernel for nn_Net_76622216561354 (gnn_message_passing).

Self-contained: host-side sharding/index prep (numpy) + an 8-core SPMD
Bass/Tile kernel run via run_bass_kernel_spmd. Accepts FULL inputs, returns
the FULL pooled output [8192] float32.

v2: compact shipped inputs (16x less relay traffic), destination-chunk
segmented edge streams (small cum segments per ends gather, gathered once
per chunk per iteration), f32-granule gathers, batched table gathers.

v3: slab exchanged as [16, NPAD] (AllGather -> one contiguous DMA into an
un-paired SBUF table, killing the 8x stage+depair copies), src gathers use
f32 pair granules + per-edge parity select, double-buffered ends pool so
the D gathers overlap the gate math.

v4: Shared-space AllGather outputs (fast HBM-HBM path, one tensor per
writer), ends-gathers interleaved into the C stream as soon as their cum
segments are complete, the next iteration's slab + partial AllGathers
chained behind each D chunk's gate update (collective hidden behind gate
math), slab staged and DMA'd once per part, per-layer gate weights
double-buffered with prefetch.
"""
import numpy as np
import concourse.bass as bass
import concourse.mybir as mybir
import concourse.tile as tile
from concourse import bacc
from contextlib import ExitStack
import os

NC = 8
N = 131072; E = 524288; F_IN = 16; DIM = 64; DNN = 16; BK = 4; NG = 8192
NL1 = 4; NL2 = 2
SUB = 2112
NPAD = 8 * SUB          # 16896
HALF = NPAD // 2        # 8448
ECH = 512
GSLOT = 192             # pooled graph slots per sub-chunk (padded)
NITER = NL1 * NL2       # 8


def _wrapw(seq):
    """seq[L] -> [16, L/16] with out[p, s] = seq[s*16 + p]."""
    L = len(seq)
    assert L % 16 == 0
    return np.asarray(seq).reshape(L // 16, 16).T.copy()


def host_prep(inputs):
    ei = np.asarray(inputs["edge_index"])
    batch = np.asarray(inputs["batch"]).astype(np.int64)
    src, dst = ei[0].astype(np.int64), ei[1].astype(np.int64)

    # ---- graph spans ----
    gsizes = np.bincount(batch, minlength=NG)
    gstart = np.concatenate([[0], np.cumsum(gsizes)])

    # ---- core cuts at graph boundaries ----
    cuts = [0]
    for c in range(1, NC):
        t = c * (N // NC)
        while t < N and batch[t] == batch[t - 1]:
            t += 1
        cuts.append(t)
    cuts.append(N)
    cuts = np.array(cuts, np.int64)

    # ---- per-core: pack graphs into 8 graph-aligned sub-chunks ----
    g2l = np.full(N, -1, np.int64)
    node_core = np.zeros(N, np.int64)
    l2g = [np.full(NPAD, -1, np.int64) for _ in range(NC)]
    pool_graphs = [[[] for _ in range(8)] for _ in range(NC)]
    pool_mask = [np.zeros((8, SUB), np.float32) for _ in range(NC)]

    for c in range(NC):
        lo, hi = cuts[c], cuts[c + 1]
        glo, ghi = batch[lo], (batch[hi - 1] + 1 if hi > lo else batch[lo])
        s = 0; pos = 0
        for g in range(glo, ghi):
            sz = int(gsizes[g])
            if sz == 0:
                continue
            if pos + sz > SUB:
                s += 1; pos = 0
                assert s < 8, f"core {c}: sub-chunk overflow"
                assert sz <= SUB
            nodes = np.arange(gstart[g], gstart[g] + sz)
            slots = s * SUB + pos + np.arange(sz)
            g2l[nodes] = slots
            node_core[nodes] = c
            l2g[c][slots] = nodes
            pool_mask[c][s, pos + 1: pos + sz] = 1.0
            pool_graphs[c][s].append((g, pos + sz - 1))
            pos += sz
        assert hi == lo or batch[hi - 1] + 1 == ghi

    dstslot = g2l[dst]; srcslot = g2l[src]
    dst_core = node_core[dst]; src_core = node_core[src]
    dstch = dstslot // SUB                      # destination sub-chunk 0..7

    # ---- segment size: per (core, block, chunk) stream + 1 dummy ----
    key = (dst_core * NC + src_core) * 8 + dstch
    counts = np.bincount(key, minlength=NC * NC * 8)
    SEGQ = 256
    SEG = int((counts.max() + 1 + SEGQ - 1) // SEGQ) * SEGQ
    EPT = 8 * SEG

    indeg = np.bincount(dst, minlength=N).astype(np.float64)
    inv = 1.0 / np.maximum(indeg, 1.0)
    ea_all = np.asarray(inputs["edge_attr"]).astype(np.float64)

    in_maps = []
    for c in range(NC):
        gidx = np.zeros((8, EPT), np.int64)
        craw = np.zeros((8, EPT, 7), np.float64)
        ends = np.zeros((8, NPAD), np.int64)    # relative to segment base
        for b in range(NC):
            m = (dst_core == c) & (src_core == b)
            eids = np.nonzero(m)[0]
            order = np.argsort(dstslot[eids], kind="stable")
            eids = eids[order]
            dsl = dstslot[eids]
            chs = dsl // SUB
            bounds = np.searchsorted(chs, np.arange(9))
            for ch in range(8):
                sl = slice(bounds[ch], bounds[ch + 1])
                e_ch = eids[sl]
                k = len(e_ch)
                if k == 0:
                    continue
                base = ch * SEG
                ps = base + 1 + np.arange(k)
                gidx[b, ps] = srcslot[e_ch] >> 1      # f32 pair-granule index
                craw[b, ps, 0] = inv[dst[e_ch]]
                for q in range(4):
                    craw[b, ps, 1 + q] = inv[dst[e_ch]] * ea_all[e_ch, q]
                d_ch = dsl[sl]
                same = np.zeros(k)
                same[1:] = d_ch[1:] == d_ch[:-1]
                craw[b, ps, 5] = same
                craw[b, ps, 6] = srcslot[e_ch] & 1    # parity within granule
                ends[b, d_ch] = ps - base       # last write wins (sorted)

        gidx_t = np.zeros((128, EPT // 16), np.int16)
        eidx_t = np.zeros((128, NPAD // 16), np.int16)
        for b in range(NC):
            gidx_t[16 * b:16 * (b + 1)] = _wrapw(gidx[b]).astype(np.int16)
            eidx_t[16 * b:16 * (b + 1)] = _wrapw(ends[b]).astype(np.int16)

        pidx_t = np.zeros((128, GSLOT // 16), np.int16)
        for s in range(8):
            seq = np.zeros(GSLOT, np.int64)
            gl = pool_graphs[c][s]
            assert len(gl) <= GSLOT, f"GSLOT overflow: {len(gl)}"
            for i, (g, endpos) in enumerate(gl):
                seq[i] = endpos
            pidx_t[16 * s:16 * (s + 1)] = _wrapw(seq).astype(np.int16)

        xT = np.zeros((16, NPAD), np.float16)
        real = l2g[c] >= 0
        xT[:, real] = np.asarray(inputs["x"])[l2g[c][real]].T.astype(np.float16)

        in_maps.append(dict(
            xT=xT,
            gidx=gidx_t,
            eidx=eidx_t,
            craw=craw.astype(np.float16),
            pmask=pool_mask[c].astype(np.float16),
            pidx=pidx_t,
        ))

    meta = dict(SEG=SEG, EPT=EPT, cuts=cuts, pool_graphs=pool_graphs, l2g=l2g)
    return in_maps, meta


def fold_weights_host(inputs):
    """float64 weight folds -> compact shipped tensors (per-core identical)."""
    dt = np.float64
    lin0_w = np.asarray(inputs["lin0_w"], dt); lin0_b = np.asarray(inputs["lin0_b"], dt)
    lin1_w = np.asarray(inputs["lin1_w"], dt); lin1_b = np.asarray(inputs["lin1_b"], dt)
    lin2_w = np.asarray(inputs["lin2_w"], dt)
    root_w = np.asarray(inputs["root_w"], dt); conv_b = np.asarray(inputs["conv_b"], dt)
    nn1_w = np.asarray(inputs["nn1_w"], dt); nn1_b = np.asarray(inputs["nn1_b"], dt)
    gw_ih = np.asarray(inputs["gru_w_ih"], dt); gw_hh = np.asarray(inputs["gru_w_hh"], dt)
    gb_ih = np.asarray(inputs["gru_b_ih"], dt); gb_hh = np.asarray(inputs["gru_b_hh"], dt)

    Bm = nn1_b.reshape(DNN, DNN)
    Ak = nn1_w.reshape(BK, DNN, DNN)
    M = np.concatenate([Bm[None], Ak], axis=0)            # [5,16,16]

    w = {}
    w["lin1w"] = lin1_w.astype(np.float16)                # [64,16]
    w["Mc"] = M.astype(np.float16)                        # [5,16,16]

    whsc = np.zeros((NL1 * 4, 64, 64), np.float64)
    wfoldc = np.zeros((NL1 * 3, 16, 64), np.float64)
    biasc = np.zeros((64, 17), np.float64)
    for j in range(NL1):
        P = lin1_w @ root_w @ gw_ih[j].T                  # [64,192]
        W_rz = P[:, :2 * DIM] + gw_hh[j].T[:, :2 * DIM]
        W_ni = P[:, 2 * DIM:]
        W_nh = gw_hh[j].T[:, 2 * DIM:]
        grp_w = [W_rz[:, :64], W_rz[:, 64:], W_ni, W_nh]
        for g in range(4):
            whsc[4 * j + g] = grp_w[g]
        wihT = gw_ih[j].T                                  # [16,192]
        for g in range(3):
            wfoldc[3 * j + g] = wihT[:, 64 * g:64 * (g + 1)]
        b_base = (lin1_b @ root_w + conv_b) @ gw_ih[j].T   # [192]
        b_rz = b_base[:2 * DIM] + gb_ih[j][:2 * DIM] + gb_hh[j][:2 * DIM]
        b_ni = b_base[2 * DIM:] + gb_ih[j][2 * DIM:]
        b_hn = gb_hh[j][2 * DIM:]
        vec = [b_rz[:64], b_rz[64:], b_ni, b_hn]
        for g in range(4):
            biasc[:, 4 * j + g] = vec[g]
    biasc[:, 16] = lin0_b
    w["whsc"] = whsc.astype(np.float16)                   # [16,64,64]
    w["wfoldc"] = wfoldc.astype(np.float16)               # [12,16,64]
    w["biasc"] = biasc.astype(np.float32)                 # [64,17]
    w["lin0w"] = lin0_w.astype(np.float16)                # [16,64]
    w["lin2w"] = lin2_w.astype(np.float16)                # [64,1]
    return w


# ================= blob packing =================
# All per-core inputs travel in ONE int16 tensor: the PJRT relay charges
# ~1.3 ms per operand buffer per execution, so operand count dominates.

def blob_layout(EPT):
    off = {}
    o = 0
    def add(name, units):
        nonlocal o
        off[name] = o
        o += units
    add("xt", 16 * NPAD)
    add("gidx", 128 * (EPT // 16))
    add("eidx", 128 * (NPAD // 16))
    add("craw", 8 * EPT * 7)
    add("pmask", 8 * SUB)
    add("pidx", 128 * (GSLOT // 16))
    add("lin1w", 64 * 16)
    add("mc", 5 * 256)
    add("whsc", 16 * 64 * 64)
    add("wfoldc", 12 * 16 * 64)
    add("lin0w", 16 * 64)
    add("lin2w", 64)
    add("biasc", 64 * 17 * 2)
    off["_total"] = o + (o & 1)
    return off


def build_in_maps(in_maps_data, w, EPT):
    off = blob_layout(EPT)
    out = []
    for c in range(NC):
        f = in_maps_data[c]
        blob = np.zeros(off["_total"], np.int16)
        def put(name, arr):
            v = np.ascontiguousarray(arr).view(np.int16).ravel()
            blob[off[name]:off[name] + v.size] = v
        put("xt", f["xT"]); put("gidx", f["gidx"]); put("eidx", f["eidx"])
        put("craw", f["craw"]); put("pmask", f["pmask"]); put("pidx", f["pidx"])
        put("lin1w", w["lin1w"]); put("mc", w["Mc"]); put("whsc", w["whsc"])
        put("wfoldc", w["wfoldc"]); put("lin0w", w["lin0w"])
        put("lin2w", w["lin2w"]); put("biasc", w["biasc"])
        out.append(dict(blob=blob))
    return out


# ================= kernel builder =================

f32 = mybir.dt.float32
f16 = mybir.dt.float16
i16 = mybir.dt.int16
AF = mybir.ActivationFunctionType
OP = mybir.AluOpType


def pieces(total, step):
    out = []
    off = 0
    while off < total:
        out.append((off, min(step, total - off)))
        off += step
    return out


def build(EPT, fake_collective=False, niter=NITER, num_devices=NC,
          hw_loop=False, stagger=False, inner_loops=True, skip=(),
          coll_overlap=True):
    # NOTE: hw_loop=True (For_i around the iteration body) executes correctly
    # in CoreSim and cuts the NEFF from ~9.2K to ~2.7K instructions, but the
    # collective inside the loop desyncs the 8-core mesh in this runtime
    # ("mesh desynced"), so the shipped configuration keeps iterations
    # unrolled.
    SEG = EPT // 8
    nchunk = EPT // ECH
    OFF = blob_layout(EPT)
    nc = bacc.Bacc("TRN2", target_bir_lowering=False, debug=False,
                   num_devices=num_devices)

    blob_d = nc.dram_tensor("blob", [OFF["_total"]], i16, kind="ExternalInput")
    out_d = nc.dram_tensor("pooled", [8, GSLOT], f32, kind="ExternalOutput")

    def bap(name, extra_off, dims, dt=f16):
        ap = bass.AP(blob_d, OFF[name] + extra_off, dims)
        return ap if dt == i16 else ap.bitcast(dt)

    PIECES_S = pieces(SUB, 512)                 # within one sub-chunk
    PIECES_H = pieces(HALF, 512)                # lin0 / final only
    GSPANS = pieces(EPT, 2048)                  # table-gather batching

    with tile.TileContext(nc) as tc, ExitStack() as ex:
        pp = ex.enter_context(tc.tile_pool(name="persist", bufs=1))
        wk = ex.enter_context(tc.tile_pool(name="work", bufs=2))
        wk2 = ex.enter_context(tc.tile_pool(name="work2", bufs=2))
        ebp = ex.enter_context(tc.tile_pool(name="ends", bufs=2))
        ps = ex.enter_context(tc.tile_pool(name="psum", bufs=6, space="PSUM"))
        dr = ex.enter_context(tc.tile_pool(name="dram", bufs=1, space="DRAM"))

        BUFA = dict(tag="bufA")
        BUFB = dict(tag="bufB")
        GGT = dict(tag="gg")

        def dyn(ap, expr):
            """Copy `ap` with `expr` (int or register) added to its offset."""
            ap = ap.copy()
            ap.offset = expr + ap.offset
            return ap

        hT = pp.tile([128, HALF], f16, tag="hT")
        # un-paired gather table: partition 16b+f = core b's feature f,
        # free dim = node slot; refreshed wholesale by the AllGather landing
        table = pp.tile([128, NPAD], f16, tag="table")
        nc.vector.memset(table[:], 0)
        ccP = None
        if 'cdma' in skip:
            ccP = pp.tile([128, ECH * 7], f16, tag="ccP")
            nc.vector.memset(ccP[:], 0)
        # leading dummy pair: scan carry-in for chunk k reads position k*ECH
        # (position 0 = zero dummy), so no k==0 special case inside loops
        cum = pp.tile([128, EPT + 1, 2], f16, tag="cum")
        nc.vector.memset(cum[:], 0)
        gbufs = [pp.tile([128, 2048, 2], f16, tag="gbufA", name="gbufA"),
                 pp.tile([128, 2048, 2], f16, tag="gbufB", name="gbufB")]
        if 'cgather' in skip:
            nc.vector.memset(gbufs[0][:], 0)
            nc.vector.memset(gbufs[1][:], 0)
        gidx = pp.tile([128, EPT // 16], i16, tag="gidx")
        eidx = pp.tile([128, NPAD // 16], i16, tag="eidx")
        pmask = pp.tile([128, SUB], f16, tag="pmask")
        pidx = pp.tile([128, GSLOT // 16], i16, tag="pidx")
        biases = pp.tile([128, 17], f32, tag="biases")
        wy = pp.tile([128, 2], f16, tag="wy")

        nc.sync.dma_start(out=gidx[:], in_=bap(
            "gidx", 0, [(EPT // 16, 128), (1, EPT // 16)], i16))
        nc.sync.dma_start(out=eidx[:], in_=bap(
            "eidx", 0, [(NPAD // 16, 128), (1, NPAD // 16)], i16))
        nc.sync.dma_start(out=pidx[:], in_=bap(
            "pidx", 0, [(GSLOT // 16, 128), (1, GSLOT // 16)], i16))
        # broadcast [8, SUB] -> [128, SUB]
        nc.sync.dma_start(
            out=pmask[:],
            in_=bap("pmask", 0, [(SUB, 8), (0, 16), (1, SUB)]))
        # broadcast [64, 17] -> [128, 17]
        nc.sync.dma_start(
            out=biases[:],
            in_=bap("biasc", 0, [(0, 2), (34, 64), (1, 34)], f32))

        # ---- stationaries assembled on device from compact inputs ----
        # lin1 stationaries for the slab: half hh contracts hT rows
        # [64*hh, 64*hh+64) into a 16-row output
        wlin1h = pp.tile([128, 2, 16], f16, tag="wlin1h")
        nc.vector.memset(wlin1h[:], 0)
        nc.sync.dma_start(out=wlin1h[0:64, 0, :],
                          in_=bap("lin1w", 0, [(16, 64), (1, 16)]))
        nc.sync.dma_start(out=wlin1h[64:128, 1, :],
                          in_=bap("lin1w", 0, [(16, 64), (1, 16)]))
        wM_s = pp.tile([128, 5, 128], f16, tag="wM_s")
        nc.vector.memset(wM_s[:], 0)
        for q in range(5):
            for b in range(8):
                nc.sync.dma_start(
                    out=wM_s[16 * b:16 * (b + 1), q, 16 * b:16 * (b + 1)],
                    in_=bap("mc", q * 256, [(16, 16), (1, 16)]))
        # per-layer gate weights, double-buffered so the next layer's DMAs
        # prefetch while the current layer's iterations run
        whs_curs = [pp.tile([128, 4, 128], f16, tag=f"whs{s}", name=f"whs{s}")
                    for s in range(2)]
        nc.vector.memset(whs_curs[0][:], 0)
        nc.vector.memset(whs_curs[1][:], 0)
        wfold_curs = [pp.tile([128, 3, 64], f16, tag=f"wfold{s}",
                              name=f"wfold{s}") for s in range(2)]
        bias_curs = [pp.tile([128, 4], f32, tag=f"biasl{s}",
                             name=f"biasl{s}") for s in range(2)]
        CUR = {}

        def bapd(name, dyn_off, dims, dt=f16):
            """blob AP with a (possibly register) offset, in target-dtype units."""
            ap = bass.AP(blob_d, 0, dims)
            if dt != i16:
                ap = ap.bitcast(dt)
            base = OFF[name] // 2 if dt == f32 else OFF[name]
            ap.offset = dyn_off + base
            return ap

        def refresh_layer(jexpr, slot):
            whs_cur = whs_curs[slot]
            wfold_cur = wfold_curs[slot]
            bias_cur = bias_curs[slot]
            for g in range(4):
                for h in range(2):
                    nc.sync.dma_start(
                        out=whs_cur[64 * h:64 * (h + 1), g, 64 * h:64 * (h + 1)],
                        in_=bapd("whsc", jexpr * (4 * 64 * 64) + g * 64 * 64,
                                 [(64, 64), (1, 64)]))
            for g in range(3):
                nc.sync.dma_start(
                    out=wfold_cur[:, g, :],
                    in_=bapd("wfoldc", jexpr * (3 * 16 * 64) + g * 16 * 64,
                             [(0, 8), (64, 16), (1, 64)]))
            nc.sync.dma_start(
                out=bias_cur[:],
                in_=bapd("biasc", jexpr * 4, [(0, 2), (34, 64), (1, 8)], f32))
        wlin0 = pp.tile([16, 2, 128], f16, tag="wlin0")
        nc.vector.memset(wlin0[:], 0)
        nc.sync.dma_start(out=wlin0[:, 0, 0:64],
                          in_=bap("lin0w", 0, [(64, 16), (1, 64)]))
        nc.sync.dma_start(out=wlin0[:, 1, 64:128],
                          in_=bap("lin0w", 0, [(64, 16), (1, 64)]))
        nc.vector.memset(wy[:], 0)
        nc.sync.dma_start(out=wy[0:64, 0:1],
                          in_=bap("lin2w", 0, [(1, 64), (1, 1)]))
        nc.sync.dma_start(out=wy[64:128, 1:2],
                          in_=bap("lin2w", 0, [(1, 64), (1, 1)]))

        slab_dram = dr.tile([16, NPAD], f16)
        # Shared-space AllGather outputs (fast HBM-HBM path); a Shared tensor
        # may only have a single writer, so one per iteration (and per part
        # in the overlapped scheme)
        if coll_overlap:
            agp_drams = [[dr.tile([NC, 16, 2, SUB], f16, addr_space="Shared",
                                  name=f"ag{i}_{p}") for p in range(4)]
                         for i in range(niter)]
            slabp_dram = dr.tile([4, 16, 2, SUB], f16)
        else:
            ag_drams = [dr.tile([NC, 16, NPAD], f16, addr_space="Shared",
                                name=f"ag{i}") for i in range(niter)]
        y_dram = dr.tile([2, HALF], f32)
        # pre-expanded edge coefficients: the 8->128 partition broadcast DMA
        # re-reads each small HBM row 16x, which is slow — do it ONCE here and
        # stream linearly every iteration
        cexp_dram = dr.tile([nchunk, 128, ECH * 7], f16)
        if 'cexp' not in skip:
            for k in range(nchunk):
                nc.sync.dma_start(
                    out=cexp_dram[k],
                    in_=bap("craw", k * (ECH * 7),
                            [(EPT * 7, 8), (0, 16), (1, ECH * 7)]))

        # ================= INIT: lin0 -> hT =================
        for c0, L in PIECES_H:
            xa = wk2.tile([16, 512], f16, **GGT)
            nc.sync.dma_start(out=xa[:, :L],
                              in_=bap("xt", c0, [(NPAD, 16), (1, L)]))
            xb = wk2.tile([16, 512], f16, **GGT)
            nc.sync.dma_start(out=xb[:, :L],
                              in_=bap("xt", HALF + c0, [(NPAD, 16), (1, L)]))
            p0 = ps.tile([128, 512], f32, tag="ps")
            nc.tensor.matmul(p0[:, :L], wlin0[:, 0, :], xa[:, :L],
                             start=True, stop=False)
            nc.tensor.matmul(p0[:, :L], wlin0[:, 1, :], xb[:, :L],
                             start=False, stop=True)
            nc.scalar.activation(out=hT[:, c0:c0 + L], in_=p0[:, :L],
                                 func=AF.Relu, bias=biases[:, 16:17], scale=1.0)

        # ================= ITERATIONS =================
        skip_slab = 'astage' in skip or 'slab' in skip
        skip_coll = 'astage' in skip or 'coll' in skip
        skip_land = 'astage' in skip or 'xstage' in skip

        def slab_part(p):
            """lin1 of sub-chunks p and p+4 (hT cols [p*SUB, +SUB), both
            partition halves) -> slabp_dram[p] (overlapped-collective mode)."""
            stg = wk2.tile([16, 2, SUB], f16, tag="slabstg2", bufs=1)
            for hh in range(2):
                for off, L in PIECES_S:
                    p0 = ps.tile([16, 512], f32, tag="ps16", bufs=2)
                    nc.tensor.matmul(p0[:, :L], wlin1h[:, hh, :],
                                     hT[:, p * SUB + off:p * SUB + off + L],
                                     start=True, stop=True)
                    nc.vector.tensor_copy(out=stg[:, hh, off:off + L],
                                          in_=p0[:, :L])
            nc.sync.dma_start(out=slabp_dram[p], in_=stg[:])

        def ag_part(it, p):
            agp = agp_drams[it][p]
            if fake_collective:
                # single 8-fold broadcast DMA (Shared tensors allow only one
                # writer instruction); traffic-equivalent to the collective
                nc.sync.dma_start(
                    out=agp[:],
                    in_=bass.AP(slabp_dram.tensor,
                                slabp_dram[p].offset,
                                [(0, 8), (2 * SUB, 16), (1, 2 * SUB)]))
            else:
                nc.gpsimd.collective_compute(
                    "AllGather", OP.bypass,
                    replica_groups=[list(range(NC))],
                    ins=[slabp_dram[p][:].opt()], outs=[agp[:].opt()])

        def emit_slab_ag(it):
            if not skip_slab:
                for p in range(4):
                    slab_part(p)
            if not skip_coll:
                for p in range(4):
                    ag_part(it, p)

        def one_iter(it):
            ag_dram = None if coll_overlap else ag_drams[it]
            # ---- A: slab (lin1 of local nodes, [16, NPAD]) + exchange ----
            if coll_overlap:
                if not skip_land:
                    for p in range(4):
                        agp = agp_drams[it][p]
                        for hh in range(2):
                            nc.sync.dma_start(
                                out=table[:, hh * HALF + p * SUB:
                                          hh * HALF + (p + 1) * SUB],
                                in_=bass.AP(
                                    agp.tensor,
                                    agp[:].offset + hh * SUB,
                                    [(16 * 2 * SUB, 8), (2 * SUB, 16),
                                     (1, SUB)]))
            else:
                for c0, L in PIECES_H if not skip_slab else []:
                    for hh in range(2):
                        p0 = ps.tile([16, 512], f32, tag="ps16", bufs=2)
                        nc.tensor.matmul(p0[:, :L], wlin1h[:, hh, :],
                                         hT[:, c0:c0 + L],
                                         start=True, stop=True)
                        stg = wk2.tile([16, 512], f16, tag="slabstg")
                        nc.vector.tensor_copy(out=stg[:, :L], in_=p0[:, :L])
                        nc.sync.dma_start(
                            out=slab_dram[:, hh * HALF + c0:hh * HALF + c0 + L],
                            in_=stg[:, :L])
                if not skip_coll:
                    if fake_collective:
                        for cc_ in range(NC):
                            nc.sync.dma_start(out=ag_dram[cc_],
                                              in_=slab_dram[:])
                    else:
                        nc.gpsimd.collective_compute(
                            "AllGather", OP.bypass,
                            replica_groups=[list(range(NC))],
                            ins=[slab_dram[:].opt()], outs=[ag_dram[:].opt()])
                if not skip_land:
                    # land the gathered [8, 16, NPAD] as one contiguous DMA
                    # into the un-paired table (partition 16b+f <- core b f)
                    nc.sync.dma_start(
                        out=table[:],
                        in_=bass.AP(ag_dram.tensor, ag_dram[:].offset,
                                    [(16 * NPAD, 8), (NPAD, 16), (1, NPAD)]))

            # ---- C: edge chunks (groups of 4, one table gather per group) ----
            ngroups = EPT // 2048

            def c_group(gv):
                gbuf = gbufs[gv % 2]
                if 'cgather' not in skip:
                    nc.gpsimd.ap_gather(
                        out_ap=gbuf[:].bitcast(f32),
                        in_ap=table[:].bitcast(f32),
                        idxs_ap=dyn(gidx[:, 0:128], gv * 128),
                        channels=128, num_elems=NPAD // 2, d=1, num_idxs=2048)
                for i in range(4):
                    if 'cdma' in skip:
                        cc = ccP
                    else:
                        cc = wk.tile([128, ECH * 7], f16, **BUFA)
                        nc.sync.dma_start(
                            out=cc[:],
                            in_=dyn(cexp_dram[i], gv * (4 * 128 * ECH * 7)))
                    # parity select: xs = par ? odd : even
                    g_even = bass.AP(gbuf.tensor,
                                     gbuf[:].offset + i * ECH * 2,
                                     [gbuf[:].ap[0], (2, ECH)])
                    g_odd = bass.AP(gbuf.tensor,
                                    gbuf[:].offset + i * ECH * 2 + 1,
                                    [gbuf[:].ap[0], (2, ECH)])
                    c_par = bass.AP(cc.tensor, cc[:].offset + 6,
                                    [cc[:].ap[0], (7, ECH)])
                    xs = wk2.tile([128, ECH], f16, tag="xsel")
                    if 'cvec' not in skip:
                        nc.vector.tensor_copy(out=xs[:], in_=g_even)
                        nc.vector.copy_predicated(out=xs[:],
                                                  mask=c_par.bitcast(i16),
                                                  data=g_odd)
                    sc = wk.tile([128, 5, ECH], f16, **BUFB)
                    x_in0 = bass.AP(xs.tensor, xs[:].offset,
                                    [xs[:].ap[0], (0, 5), (1, ECH)])
                    c_in1 = bass.AP(cc.tensor, cc[:].offset,
                                    [cc[:].ap[0], (1, 5), (7, ECH)])
                    if 'cvec' not in skip:
                        nc.vector.tensor_tensor(out=sc[:], in0=x_in0,
                                                in1=c_in1, op=OP.mult)
                    msg = ps.tile([128, 512], f32, tag="ps")
                    if 'cmm' not in skip:
                        for p in range(5):
                            nc.tensor.matmul(msg[:, :ECH], wM_s[:, p, :],
                                             sc[:, p, :],
                                             start=(p == 0), stop=(p == 4))
                    cum_out = dyn(bass.AP(
                        cum.tensor, cum[:].offset + (1 + i * ECH) * 2,
                        [cum[:].ap[0], (2, ECH)]), gv * 4096)
                    init = dyn(bass.AP(
                        cum.tensor, cum[:].offset + i * ECH * 2,
                        [cum[:].ap[0], (1, 1)]), gv * 4096)
                    c_mask = bass.AP(cc.tensor, cc[:].offset + 5,
                                     [cc[:].ap[0], (7, ECH)])
                    if 'cscan' not in skip:
                        nc.vector.tensor_tensor_scan(out=cum_out, data0=c_mask,
                                                     data1=msg[:, :ECH],
                                                     initial=init,
                                                     op0=OP.mult, op1=OP.add)

            # ---- D+E: ends + gates, aligned to sub-chunks ----
            ebs_store = {}

            def d_gather(chv, half):
                eb = ebp.tile([128, SUB, 2], f16,
                              tag=("ebA" if half == 0 else "ebB"))
                if 'dgather' in skip:
                    nc.vector.memset(eb[:], 0)
                else:
                    nc.gpsimd.ap_gather(
                        out_ap=eb[:].bitcast(f32),
                        in_ap=dyn(cum[:, 1:1 + SEG, :].bitcast(f32),
                                  chv * SEG + half * (4 * SEG)),
                        idxs_ap=dyn(
                            eidx[:, 0:SUB // 16],
                            chv * (SUB // 16) + half * (4 * (SUB // 16))),
                        channels=128, num_elems=SEG, d=1, num_idxs=SUB)
                ebs_store[(chv, half)] = eb

            def d_gates(chv):
                whs_cur = CUR['whs']
                wfold_cur = CUR['wfold']
                bias_cur = CUR['bias']
                ebs = [ebs_store.pop((chv, 0)), ebs_store.pop((chv, 1))]
                for off, L in PIECES_S:
                    pr = ps.tile([128, 512], f32, tag="ps")
                    pz = ps.tile([128, 512], f32, tag="ps")
                    pn = ps.tile([128, 512], f32, tag="ps")
                    ph = ps.tile([128, 512], f32, tag="ps")
                    if 'dmm' not in skip:
                        for g, pst in enumerate([pr, pz, pn, ph]):
                            nc.tensor.matmul(pst[:, :L], whs_cur[:, g, :],
                                             dyn(hT[:, off:off + L], chv * SUB),
                                             start=True, stop=True)
                        for g, pst in enumerate([pr, pz, pn]):
                            for half in range(2):
                                eb = ebs[half]
                                rhs = bass.AP(eb.tensor, eb[:].offset + off * 2,
                                              [eb[:].ap[0], (2, L)])
                                out = bass.AP(
                                    pst.tensor,
                                    pst[:].offset + 64 * half * pst[:].ap[0][0],
                                    [(pst[:].ap[0][0], 64), (1, L)])
                                tp = (0, 64) if half == 1 else None
                                nc.tensor.matmul(out, wfold_cur[:, g, :], rhs,
                                                 start=False, stop=False,
                                                 skip_group_check=True,
                                                 tile_position=tp)
                    if 'dvec' in skip:
                        continue
                    r16 = wk2.tile([128, 512], f16, tag="g_r")
                    z16 = wk2.tile([128, 512], f16, tag="g_z")
                    nc.scalar.activation(out=r16[:, :L], in_=pr[:, :L],
                                         func=AF.Sigmoid,
                                         bias=bias_cur[:, 0:1], scale=1.0)
                    nc.scalar.activation(out=z16[:, :L], in_=pz[:, :L],
                                         func=AF.Sigmoid,
                                         bias=bias_cur[:, 1:2],
                                         scale=1.0)
                    t16 = wk2.tile([128, 512], f16, tag="g_t")
                    nc.vector.scalar_tensor_tensor(
                        out=t16[:, :L], in0=ph[:, :L],
                        scalar=bias_cur[:, 3:4], in1=r16[:, :L],
                        op0=OP.add, op1=OP.mult)
                    u16 = wk2.tile([128, 512], f16, tag="g_u")
                    nc.vector.tensor_tensor(out=u16[:, :L], in0=t16[:, :L],
                                            in1=pn[:, :L], op=OP.add)
                    n16 = wk2.tile([128, 512], f16, tag="g_n")
                    nc.scalar.activation(out=n16[:, :L], in_=u16[:, :L],
                                         func=AF.Tanh,
                                         bias=bias_cur[:, 2:3],
                                         scale=1.0)
                    v16 = wk2.tile([128, 512], f16, tag="g_t")
                    nc.vector.tensor_tensor(out=v16[:, :L],
                                            in0=dyn(hT[:, off:off + L],
                                                    chv * SUB),
                                            in1=n16[:, :L], op=OP.subtract)
                    w16 = wk2.tile([128, 512], f16, tag="g_u")
                    nc.vector.tensor_tensor(out=w16[:, :L], in0=z16[:, :L],
                                            in1=v16[:, :L], op=OP.mult)
                    nc.vector.tensor_tensor(out=dyn(hT[:, off:off + L],
                                                    chv * SUB),
                                            in0=n16[:, :L],
                                            in1=w16[:, :L], op=OP.add)

            def d_chunk(chv):
                d_gather(chv, 0)
                d_gather(chv, 1)
                d_gates(chv)

            def after_gates(chv):
                if coll_overlap and it + 1 < niter:
                    if not skip_slab:
                        slab_part(chv)
                    if not skip_coll:
                        ag_part(it + 1, chv)

            interleave = (SEG == 1280 and ngroups == 5 and 'cstage' not in skip
                          and 'dstage' not in skip)
            if interleave:
                # weave the ends-gathers (Pool engine) into the C stream as
                # soon as their cum segments are complete, so they overlap the
                # C-side DVE/PE work instead of serializing after it; gather
                # needs groups 0..ceil((chv+1+4*half)*SEG/2048)-1 done
                c_group(0)
                c_group(1)
                d_gather(0, 0)          # pos < 1280  <= g0
                c_group(2)
                d_gather(1, 0)          # pos < 2560  <= g1
                c_group(3)
                d_gather(0, 1)          # pos < 6400  <= g3
                d_gather(1, 1)          # pos < 7680  <= g3
                c_group(4)
                d_gates(0)
                after_gates(0)
                d_gather(2, 0)
                d_gather(2, 1)
                d_gates(1)
                after_gates(1)
                d_gather(3, 0)
                d_gather(3, 1)
                d_gates(2)
                after_gates(2)
                d_gates(3)
                after_gates(3)
                return

            if 'cstage' in skip:
                pass
            elif inner_loops:
                with tc.For_i(0, ngroups) as gv:
                    c_group(gv)
            else:
                for gv in range(ngroups):
                    c_group(gv)
            if 'dstage' in skip:
                pass
            elif inner_loops:
                with tc.For_i(0, 4) as chv:
                    d_chunk(chv)
            else:
                for chv in range(4):
                    d_chunk(chv)
            if coll_overlap:
                for chv in range(4):
                    after_gates(chv)

        # prologue: first iteration's slab + partial AllGathers come from
        # the lin0 output directly
        if coll_overlap and niter > 0:
            emit_slab_ag(0)
        if niter > 0:
            refresh_layer(0, 0)
        for it in range(niter):
            layer = (it // 2) % NL1
            slot = layer % 2
            CUR['whs'] = whs_curs[slot]
            CUR['wfold'] = wfold_curs[slot]
            CUR['bias'] = bias_curs[slot]
            one_iter(it)
            if it % 2 == 0 and it + 2 < niter:
                # prefetch next layer's weights into the other slot while
                # this layer's second iteration runs
                refresh_layer(((it // 2) + 1) % NL1, slot ^ 1)

        # ================= FINAL: y + pooling =================
        for c0, L in PIECES_H:
            py = ps.tile([2, 512], f32, tag="ps")
            nc.tensor.matmul(py[:, :L], wy[:], hT[:, c0:c0 + L],
                             start=True, stop=True)
            ystg = wk2.tile([2, 512], f32, **GGT)
            nc.vector.tensor_copy(out=ystg[:, :L], in_=py[:, :L])
            nc.sync.dma_start(out=y_dram[:, c0:c0 + L], in_=ystg[:, :L])
        ypool = ebp.tile([128, SUB], f32, tag="ebA")
        for half in range(2):
            src = bass.AP(y_dram.tensor, y_dram[:].offset + half * HALF,
                          [(SUB, 4), (0, 16), (1, SUB)])
            nc.sync.dma_start(out=ypool[:][64 * half:64 * (half + 1)], in_=src)
        ycum = ebp.tile([128, SUB], f32, tag="ebB")
        nc.vector.tensor_tensor_scan(out=ycum[:], data0=pmask[:],
                                     data1=ypool[:], initial=0.0,
                                     op0=OP.mult, op1=OP.add)
        pooled = wk2.tile([128, GSLOT], f32, tag="g_r")
        nc.gpsimd.ap_gather(out_ap=pooled[:], in_ap=ycum[:], idxs_ap=pidx[:],
                            channels=128, num_elems=SUB, d=1, num_idxs=GSLOT)
        nc.sync.dma_start(out=out_d[:], in_=pooled[:][0::16])

    nc.compile()
    return nc


# ================= driver =================
_CACHE = {}


def kernel(**inputs):
    inputs = {k: np.asarray(v) for k, v in inputs.items()}
    in_maps_data, meta = host_prep(inputs)
    w = fold_weights_host(inputs)
    EPT = meta["EPT"]
    if EPT not in _CACHE:
        _CACHE[EPT] = build(EPT)
    nc = _CACHE[EPT]
    from concourse.bass_utils import run_bass_kernel_spmd
    in_maps = build_in_maps(in_maps_data, w, EPT)
    trace = os.environ.get("KERNEL_PROFILE", "0") == "1"
    br = run_bass_kernel_spmd(nc, in_maps, list(range(NC)), trace=trace)
    if trace and br.exec_time_ns is not None:
        print(f"HW exec time: {br.exec_time_ns} ns")
    got = np.zeros(NG, np.float32)
    for c in range(NC):
        pooled = br.results[c]["pooled"]
        for s in range(8):
            for i, (g, endpos) in enumerate(meta["pool_graphs"][c][s]):
                got[g] = pooled[s, i]
    return got



# revision 51
# speedup vs baseline: 1.1885x; 1.1885x over previous
"""Trainium2 Bass kernel for nn_Net_76622216561354 (gnn_message_passing).

Self-contained: host-side sharding/index prep (numpy) + an 8-core SPMD
Bass/Tile kernel run via run_bass_kernel_spmd. Accepts FULL inputs, returns
the FULL pooled output [8192] float32.

v2: compact shipped inputs (16x less relay traffic), destination-chunk
segmented edge streams (small cum segments per ends gather, gathered once
per chunk per iteration), f32-granule gathers, batched table gathers.

v3: slab exchanged as [16, NPAD] (AllGather -> one contiguous DMA into an
un-paired SBUF table, killing the 8x stage+depair copies), src gathers use
f32 pair granules + per-edge parity select, double-buffered ends pool so
the D gathers overlap the gate math.

v4: Shared-space AllGather outputs (fast HBM-HBM path, one tensor per
writer), ends-gathers interleaved into the C stream as soon as their cum
segments are complete, the next iteration's slab + partial AllGathers
chained behind each D chunk's gate update (collective hidden behind gate
math), slab staged and DMA'd once per part, per-layer gate weights
double-buffered with prefetch.
"""
import numpy as np
import concourse.bass as bass
import concourse.mybir as mybir
import concourse.tile as tile
from concourse import bacc
from contextlib import ExitStack
import os

NC = 8
N = 131072; E = 524288; F_IN = 16; DIM = 64; DNN = 16; BK = 4; NG = 8192
NL1 = 4; NL2 = 2
SUB = 2112
NPAD = 8 * SUB          # 16896
HALF = NPAD // 2        # 8448
ECH = 512
GSLOT = 192             # pooled graph slots per sub-chunk (padded)
NITER = NL1 * NL2       # 8


def _wrapw(seq):
    """seq[L] -> [16, L/16] with out[p, s] = seq[s*16 + p]."""
    L = len(seq)
    assert L % 16 == 0
    return np.asarray(seq).reshape(L // 16, 16).T.copy()


def host_prep(inputs):
    ei = np.asarray(inputs["edge_index"])
    batch = np.asarray(inputs["batch"]).astype(np.int64)
    src, dst = ei[0].astype(np.int64), ei[1].astype(np.int64)

    # ---- graph spans ----
    gsizes = np.bincount(batch, minlength=NG)
    gstart = np.concatenate([[0], np.cumsum(gsizes)])

    # ---- core cuts at graph boundaries ----
    cuts = [0]
    for c in range(1, NC):
        t = c * (N // NC)
        while t < N and batch[t] == batch[t - 1]:
            t += 1
        cuts.append(t)
    cuts.append(N)
    cuts = np.array(cuts, np.int64)

    # ---- per-core: pack graphs into 8 graph-aligned sub-chunks ----
    g2l = np.full(N, -1, np.int64)
    node_core = np.zeros(N, np.int64)
    l2g = [np.full(NPAD, -1, np.int64) for _ in range(NC)]
    pool_graphs = [[[] for _ in range(8)] for _ in range(NC)]
    pool_mask = [np.zeros((8, SUB), np.float32) for _ in range(NC)]

    for c in range(NC):
        lo, hi = cuts[c], cuts[c + 1]
        glo, ghi = batch[lo], (batch[hi - 1] + 1 if hi > lo else batch[lo])
        s = 0; pos = 0
        for g in range(glo, ghi):
            sz = int(gsizes[g])
            if sz == 0:
                continue
            if pos + sz > SUB:
                s += 1; pos = 0
                assert s < 8, f"core {c}: sub-chunk overflow"
                assert sz <= SUB
            nodes = np.arange(gstart[g], gstart[g] + sz)
            slots = s * SUB + pos + np.arange(sz)
            g2l[nodes] = slots
            node_core[nodes] = c
            l2g[c][slots] = nodes
            pool_mask[c][s, pos + 1: pos + sz] = 1.0
            pool_graphs[c][s].append((g, pos + sz - 1))
            pos += sz
        assert hi == lo or batch[hi - 1] + 1 == ghi

    dstslot = g2l[dst]; srcslot = g2l[src]
    dst_core = node_core[dst]; src_core = node_core[src]
    dstch = dstslot // SUB                      # destination sub-chunk 0..7

    # ---- segment size: per (core, block, chunk) stream + 1 dummy ----
    key = (dst_core * NC + src_core) * 8 + dstch
    counts = np.bincount(key, minlength=NC * NC * 8)
    SEGQ = 256
    SEG = int((counts.max() + 1 + SEGQ - 1) // SEGQ) * SEGQ
    EPT = 8 * SEG

    indeg = np.bincount(dst, minlength=N).astype(np.float64)
    inv = 1.0 / np.maximum(indeg, 1.0)
    ea_all = np.asarray(inputs["edge_attr"]).astype(np.float64)

    in_maps = []
    for c in range(NC):
        gidx = np.zeros((8, EPT), np.int64)
        craw = np.zeros((8, EPT, 7), np.float64)
        ends = np.zeros((8, NPAD), np.int64)    # relative to segment base
        for b in range(NC):
            m = (dst_core == c) & (src_core == b)
            eids = np.nonzero(m)[0]
            order = np.argsort(dstslot[eids], kind="stable")
            eids = eids[order]
            dsl = dstslot[eids]
            chs = dsl // SUB
            bounds = np.searchsorted(chs, np.arange(9))
            for ch in range(8):
                sl = slice(bounds[ch], bounds[ch + 1])
                e_ch = eids[sl]
                k = len(e_ch)
                if k == 0:
                    continue
                base = ch * SEG
                ps = base + 1 + np.arange(k)
                gidx[b, ps] = srcslot[e_ch] >> 1      # f32 pair-granule index
                craw[b, ps, 0] = inv[dst[e_ch]]
                for q in range(4):
                    craw[b, ps, 1 + q] = inv[dst[e_ch]] * ea_all[e_ch, q]
                d_ch = dsl[sl]
                same = np.zeros(k)
                same[1:] = d_ch[1:] == d_ch[:-1]
                craw[b, ps, 5] = same
                craw[b, ps, 6] = srcslot[e_ch] & 1    # parity within granule
                ends[b, d_ch] = ps - base       # last write wins (sorted)

        gidx_t = np.zeros((128, EPT // 16), np.int16)
        eidx_t = np.zeros((128, NPAD // 16), np.int16)
        for b in range(NC):
            gidx_t[16 * b:16 * (b + 1)] = _wrapw(gidx[b]).astype(np.int16)
            eidx_t[16 * b:16 * (b + 1)] = _wrapw(ends[b]).astype(np.int16)

        pidx_t = np.zeros((128, GSLOT // 16), np.int16)
        for s in range(8):
            seq = np.zeros(GSLOT, np.int64)
            gl = pool_graphs[c][s]
            assert len(gl) <= GSLOT, f"GSLOT overflow: {len(gl)}"
            for i, (g, endpos) in enumerate(gl):
                seq[i] = endpos
            pidx_t[16 * s:16 * (s + 1)] = _wrapw(seq).astype(np.int16)

        xT = np.zeros((16, NPAD), np.float16)
        real = l2g[c] >= 0
        xT[:, real] = np.asarray(inputs["x"])[l2g[c][real]].T.astype(np.float16)

        in_maps.append(dict(
            xT=xT,
            gidx=gidx_t,
            eidx=eidx_t,
            craw=craw.astype(np.float16),
            pmask=pool_mask[c].astype(np.float16),
            pidx=pidx_t,
        ))

    meta = dict(SEG=SEG, EPT=EPT, cuts=cuts, pool_graphs=pool_graphs, l2g=l2g)
    return in_maps, meta


def fold_weights_host(inputs):
    """float64 weight folds -> compact shipped tensors (per-core identical)."""
    dt = np.float64
    lin0_w = np.asarray(inputs["lin0_w"], dt); lin0_b = np.asarray(inputs["lin0_b"], dt)
    lin1_w = np.asarray(inputs["lin1_w"], dt); lin1_b = np.asarray(inputs["lin1_b"], dt)
    lin2_w = np.asarray(inputs["lin2_w"], dt)
    root_w = np.asarray(inputs["root_w"], dt); conv_b = np.asarray(inputs["conv_b"], dt)
    nn1_w = np.asarray(inputs["nn1_w"], dt); nn1_b = np.asarray(inputs["nn1_b"], dt)
    gw_ih = np.asarray(inputs["gru_w_ih"], dt); gw_hh = np.asarray(inputs["gru_w_hh"], dt)
    gb_ih = np.asarray(inputs["gru_b_ih"], dt); gb_hh = np.asarray(inputs["gru_b_hh"], dt)

    Bm = nn1_b.reshape(DNN, DNN)
    Ak = nn1_w.reshape(BK, DNN, DNN)
    M = np.concatenate([Bm[None], Ak], axis=0)            # [5,16,16]

    w = {}
    w["lin1w"] = lin1_w.astype(np.float16)                # [64,16]
    w["Mc"] = M.astype(np.float16)                        # [5,16,16]

    whsc = np.zeros((NL1 * 4, 64, 64), np.float64)
    wfoldc = np.zeros((NL1 * 3, 16, 64), np.float64)
    biasc = np.zeros((64, 17), np.float64)
    for j in range(NL1):
        P = lin1_w @ root_w @ gw_ih[j].T                  # [64,192]
        W_rz = P[:, :2 * DIM] + gw_hh[j].T[:, :2 * DIM]
        W_ni = P[:, 2 * DIM:]
        W_nh = gw_hh[j].T[:, 2 * DIM:]
        grp_w = [W_rz[:, :64], W_rz[:, 64:], W_ni, W_nh]
        for g in range(4):
            whsc[4 * j + g] = grp_w[g]
        wihT = gw_ih[j].T                                  # [16,192]
        for g in range(3):
            wfoldc[3 * j + g] = wihT[:, 64 * g:64 * (g + 1)]
        b_base = (lin1_b @ root_w + conv_b) @ gw_ih[j].T   # [192]
        b_rz = b_base[:2 * DIM] + gb_ih[j][:2 * DIM] + gb_hh[j][:2 * DIM]
        b_ni = b_base[2 * DIM:] + gb_ih[j][2 * DIM:]
        b_hn = gb_hh[j][2 * DIM:]
        vec = [b_rz[:64], b_rz[64:], b_ni, b_hn]
        for g in range(4):
            biasc[:, 4 * j + g] = vec[g]
    biasc[:, 16] = lin0_b
    w["whsc"] = whsc.astype(np.float16)                   # [16,64,64]
    w["wfoldc"] = wfoldc.astype(np.float16)               # [12,16,64]
    w["biasc"] = biasc.astype(np.float32)                 # [64,17]
    w["lin0w"] = lin0_w.astype(np.float16)                # [16,64]
    w["lin2w"] = lin2_w.astype(np.float16)                # [64,1]
    return w


# ================= blob packing =================
# All per-core inputs travel in ONE int16 tensor: the PJRT relay charges
# ~1.3 ms per operand buffer per execution, so operand count dominates.

def blob_layout(EPT):
    off = {}
    o = 0
    def add(name, units):
        nonlocal o
        off[name] = o
        o += units
    add("xt", 16 * NPAD)
    add("gidx", 128 * (EPT // 16))
    add("eidx", 128 * (NPAD // 16))
    add("craw", 8 * EPT * 7)
    add("pmask", 8 * SUB)
    add("pidx", 128 * (GSLOT // 16))
    add("lin1w", 64 * 16)
    add("mc", 5 * 256)
    add("whsc", 16 * 64 * 64)
    add("wfoldc", 12 * 16 * 64)
    add("lin0w", 16 * 64)
    add("lin2w", 64)
    add("biasc", 64 * 17 * 2)
    off["_total"] = o + (o & 1)
    return off


def build_in_maps(in_maps_data, w, EPT):
    off = blob_layout(EPT)
    out = []
    for c in range(NC):
        f = in_maps_data[c]
        blob = np.zeros(off["_total"], np.int16)
        def put(name, arr):
            v = np.ascontiguousarray(arr).view(np.int16).ravel()
            blob[off[name]:off[name] + v.size] = v
        put("xt", f["xT"]); put("gidx", f["gidx"]); put("eidx", f["eidx"])
        put("craw", f["craw"]); put("pmask", f["pmask"]); put("pidx", f["pidx"])
        put("lin1w", w["lin1w"]); put("mc", w["Mc"]); put("whsc", w["whsc"])
        put("wfoldc", w["wfoldc"]); put("lin0w", w["lin0w"])
        put("lin2w", w["lin2w"]); put("biasc", w["biasc"])
        out.append(dict(blob=blob))
    return out


# ================= kernel builder =================

f32 = mybir.dt.float32
f16 = mybir.dt.float16
i16 = mybir.dt.int16
AF = mybir.ActivationFunctionType
OP = mybir.AluOpType


def pieces(total, step):
    out = []
    off = 0
    while off < total:
        out.append((off, min(step, total - off)))
        off += step
    return out


def build(EPT, fake_collective=False, niter=NITER, num_devices=NC,
          hw_loop=False, stagger=False, inner_loops=True, skip=(),
          coll_overlap=True):
    # NOTE: hw_loop=True (For_i around the iteration body) executes correctly
    # in CoreSim and cuts the NEFF from ~9.2K to ~2.7K instructions, but the
    # collective inside the loop desyncs the 8-core mesh in this runtime
    # ("mesh desynced"), so the shipped configuration keeps iterations
    # unrolled.
    SEG = EPT // 8
    nchunk = EPT // ECH
    OFF = blob_layout(EPT)
    nc = bacc.Bacc("TRN2", target_bir_lowering=False, debug=False,
                   num_devices=num_devices)

    blob_d = nc.dram_tensor("blob", [OFF["_total"]], i16, kind="ExternalInput")
    out_d = nc.dram_tensor("pooled", [8, GSLOT], f32, kind="ExternalOutput")

    def bap(name, extra_off, dims, dt=f16):
        ap = bass.AP(blob_d, OFF[name] + extra_off, dims)
        return ap if dt == i16 else ap.bitcast(dt)

    PIECES_S = pieces(SUB, 512)                 # within one sub-chunk
    PIECES_H = pieces(HALF, 512)                # lin0 / final only
    GSPANS = pieces(EPT, 2048)                  # table-gather batching

    with tile.TileContext(nc) as tc, ExitStack() as ex:
        pp = ex.enter_context(tc.tile_pool(name="persist", bufs=1))
        wk = ex.enter_context(tc.tile_pool(name="work", bufs=2))
        wk2 = ex.enter_context(tc.tile_pool(name="work2", bufs=2))
        ebp = ex.enter_context(tc.tile_pool(name="ends", bufs=2))
        ps = ex.enter_context(tc.tile_pool(name="psum", bufs=6, space="PSUM"))
        dr = ex.enter_context(tc.tile_pool(name="dram", bufs=1, space="DRAM"))

        BUFA = dict(tag="bufA")
        BUFB = dict(tag="bufB")
        GGT = dict(tag="gg")

        def dyn(ap, expr):
            """Copy `ap` with `expr` (int or register) added to its offset."""
            ap = ap.copy()
            ap.offset = expr + ap.offset
            return ap

        hT = pp.tile([128, HALF], f16, tag="hT")
        # un-paired gather table: partition 16b+f = core b's feature f,
        # free dim = node slot; refreshed wholesale by the AllGather landing
        table = pp.tile([128, NPAD], f16, tag="table")
        nc.vector.memset(table[:], 0)
        ccP = None
        if 'cdma' in skip:
            ccP = pp.tile([128, ECH * 7], f16, tag="ccP")
            nc.vector.memset(ccP[:], 0)
        # leading dummy pair: scan carry-in for chunk k reads position k*ECH
        # (position 0 = zero dummy), so no k==0 special case inside loops
        cum = pp.tile([128, EPT + 1, 2], f16, tag="cum")
        nc.vector.memset(cum[:], 0)
        gbufs = [pp.tile([128, 2048, 2], f16, tag="gbufA", name="gbufA"),
                 pp.tile([128, 2048, 2], f16, tag="gbufB", name="gbufB")]
        if 'cgather' in skip:
            nc.vector.memset(gbufs[0][:], 0)
            nc.vector.memset(gbufs[1][:], 0)
        gidx = pp.tile([128, EPT // 16], i16, tag="gidx")
        eidx = pp.tile([128, NPAD // 16], i16, tag="eidx")
        pmask = pp.tile([128, SUB], f16, tag="pmask")
        pidx = pp.tile([128, GSLOT // 16], i16, tag="pidx")
        biases = pp.tile([128, 17], f32, tag="biases")
        wy = pp.tile([128, 2], f16, tag="wy")

        nc.sync.dma_start(out=gidx[:], in_=bap(
            "gidx", 0, [(EPT // 16, 128), (1, EPT // 16)], i16))
        nc.sync.dma_start(out=eidx[:], in_=bap(
            "eidx", 0, [(NPAD // 16, 128), (1, NPAD // 16)], i16))
        nc.sync.dma_start(out=pidx[:], in_=bap(
            "pidx", 0, [(GSLOT // 16, 128), (1, GSLOT // 16)], i16))
        # broadcast [8, SUB] -> [128, SUB]
        nc.sync.dma_start(
            out=pmask[:],
            in_=bap("pmask", 0, [(SUB, 8), (0, 16), (1, SUB)]))
        # broadcast [64, 17] -> [128, 17]
        nc.sync.dma_start(
            out=biases[:],
            in_=bap("biasc", 0, [(0, 2), (34, 64), (1, 34)], f32))

        # ---- stationaries assembled on device from compact inputs ----
        # lin1 stationaries for the slab: half hh contracts hT rows
        # [64*hh, 64*hh+64) into a 16-row output
        wlin1h = pp.tile([128, 2, 16], f16, tag="wlin1h")
        nc.vector.memset(wlin1h[:], 0)
        nc.sync.dma_start(out=wlin1h[0:64, 0, :],
                          in_=bap("lin1w", 0, [(16, 64), (1, 16)]))
        nc.sync.dma_start(out=wlin1h[64:128, 1, :],
                          in_=bap("lin1w", 0, [(16, 64), (1, 16)]))
        wM_s = pp.tile([128, 5, 128], f16, tag="wM_s")
        nc.vector.memset(wM_s[:], 0)
        for q in range(5):
            for b in range(8):
                nc.sync.dma_start(
                    out=wM_s[16 * b:16 * (b + 1), q, 16 * b:16 * (b + 1)],
                    in_=bap("mc", q * 256, [(16, 16), (1, 16)]))
        # per-layer gate weights, double-buffered so the next layer's DMAs
        # prefetch while the current layer's iterations run
        whs_curs = [pp.tile([128, 4, 128], f16, tag=f"whs{s}", name=f"whs{s}")
                    for s in range(2)]
        nc.vector.memset(whs_curs[0][:], 0)
        nc.vector.memset(whs_curs[1][:], 0)
        wfold_curs = [pp.tile([128, 3, 64], f16, tag=f"wfold{s}",
                              name=f"wfold{s}") for s in range(2)]
        bias_curs = [pp.tile([128, 4], f32, tag=f"biasl{s}",
                             name=f"biasl{s}") for s in range(2)]
        CUR = {}

        def bapd(name, dyn_off, dims, dt=f16):
            """blob AP with a (possibly register) offset, in target-dtype units."""
            ap = bass.AP(blob_d, 0, dims)
            if dt != i16:
                ap = ap.bitcast(dt)
            base = OFF[name] // 2 if dt == f32 else OFF[name]
            ap.offset = dyn_off + base
            return ap

        def refresh_layer(jexpr, slot):
            whs_cur = whs_curs[slot]
            wfold_cur = wfold_curs[slot]
            bias_cur = bias_curs[slot]
            for g in range(4):
                for h in range(2):
                    nc.sync.dma_start(
                        out=whs_cur[64 * h:64 * (h + 1), g, 64 * h:64 * (h + 1)],
                        in_=bapd("whsc", jexpr * (4 * 64 * 64) + g * 64 * 64,
                                 [(64, 64), (1, 64)]))
            for g in range(3):
                nc.sync.dma_start(
                    out=wfold_cur[:, g, :],
                    in_=bapd("wfoldc", jexpr * (3 * 16 * 64) + g * 16 * 64,
                             [(0, 8), (64, 16), (1, 64)]))
            nc.sync.dma_start(
                out=bias_cur[:],
                in_=bapd("biasc", jexpr * 4, [(0, 2), (34, 64), (1, 8)], f32))
        wlin0 = pp.tile([16, 2, 128], f16, tag="wlin0")
        nc.vector.memset(wlin0[:], 0)
        nc.sync.dma_start(out=wlin0[:, 0, 0:64],
                          in_=bap("lin0w", 0, [(64, 16), (1, 64)]))
        nc.sync.dma_start(out=wlin0[:, 1, 64:128],
                          in_=bap("lin0w", 0, [(64, 16), (1, 64)]))
        nc.vector.memset(wy[:], 0)
        nc.sync.dma_start(out=wy[0:64, 0:1],
                          in_=bap("lin2w", 0, [(1, 64), (1, 1)]))
        nc.sync.dma_start(out=wy[64:128, 1:2],
                          in_=bap("lin2w", 0, [(1, 64), (1, 1)]))

        slab_dram = dr.tile([16, NPAD], f16)
        # Shared-space AllGather outputs (fast HBM-HBM path); a Shared tensor
        # may only have a single writer, so one per iteration (and per part
        # in the overlapped scheme)
        if coll_overlap:
            agp_drams = [[dr.tile([NC, 16, 2, SUB], f16, addr_space="Shared",
                                  name=f"ag{i}_{p}") for p in range(4)]
                         for i in range(niter)]
            slabp_dram = dr.tile([4, 16, 2, SUB], f16)
        else:
            ag_drams = [dr.tile([NC, 16, NPAD], f16, addr_space="Shared",
                                name=f"ag{i}") for i in range(niter)]
        y_dram = dr.tile([2, HALF], f32)
        # pre-expanded edge coefficients: the 8->128 partition broadcast DMA
        # re-reads each small HBM row 16x, which is slow — do it ONCE here and
        # stream linearly every iteration
        cexp_dram = dr.tile([nchunk, 128, ECH * 7], f16)
        if 'cexp' not in skip:
            for k in range(nchunk):
                nc.sync.dma_start(
                    out=cexp_dram[k],
                    in_=bap("craw", k * (ECH * 7),
                            [(EPT * 7, 8), (0, 16), (1, ECH * 7)]))

        # ================= INIT: lin0 -> hT =================
        for c0, L in PIECES_H:
            xa = wk2.tile([16, 512], f16, **GGT)
            nc.sync.dma_start(out=xa[:, :L],
                              in_=bap("xt", c0, [(NPAD, 16), (1, L)]))
            xb = wk2.tile([16, 512], f16, **GGT)
            nc.sync.dma_start(out=xb[:, :L],
                              in_=bap("xt", HALF + c0, [(NPAD, 16), (1, L)]))
            p0 = ps.tile([128, 512], f32, tag="ps")
            nc.tensor.matmul(p0[:, :L], wlin0[:, 0, :], xa[:, :L],
                             start=True, stop=False)
            nc.tensor.matmul(p0[:, :L], wlin0[:, 1, :], xb[:, :L],
                             start=False, stop=True)
            nc.scalar.activation(out=hT[:, c0:c0 + L], in_=p0[:, :L],
                                 func=AF.Relu, bias=biases[:, 16:17], scale=1.0)

        # ================= ITERATIONS =================
        skip_slab = 'astage' in skip or 'slab' in skip
        skip_coll = 'astage' in skip or 'coll' in skip
        skip_land = 'astage' in skip or 'xstage' in skip

        def slab_part(p):
            """lin1 of sub-chunks p and p+4 (hT cols [p*SUB, +SUB), both
            partition halves) -> slabp_dram[p] (overlapped-collective mode)."""
            stg = wk2.tile([16, 2, SUB], f16, tag="slabstg2", bufs=1)
            for hh in range(2):
                for off, L in PIECES_S:
                    p0 = ps.tile([16, 512], f32, tag="ps16", bufs=2)
                    nc.tensor.matmul(p0[:, :L], wlin1h[:, hh, :],
                                     hT[:, p * SUB + off:p * SUB + off + L],
                                     start=True, stop=True)
                    nc.vector.tensor_copy(out=stg[:, hh, off:off + L],
                                          in_=p0[:, :L])
            nc.sync.dma_start(out=slabp_dram[p], in_=stg[:])

        def ag_part(it, p):
            agp = agp_drams[it][p]
            if fake_collective:
                # single 8-fold broadcast DMA (Shared tensors allow only one
                # writer instruction); traffic-equivalent to the collective
                nc.sync.dma_start(
                    out=agp[:],
                    in_=bass.AP(slabp_dram.tensor,
                                slabp_dram[p].offset,
                                [(0, 8), (2 * SUB, 16), (1, 2 * SUB)]))
            else:
                nc.gpsimd.collective_compute(
                    "AllGather", OP.bypass,
                    replica_groups=[list(range(NC))],
                    ins=[slabp_dram[p][:].opt()], outs=[agp[:].opt()])

        def emit_slab_ag(it):
            if not skip_slab:
                for p in range(4):
                    slab_part(p)
            if not skip_coll:
                for p in range(4):
                    ag_part(it, p)

        def one_iter(it):
            ag_dram = None if coll_overlap else ag_drams[it]
            # ---- A: slab (lin1 of local nodes, [16, NPAD]) + exchange ----
            if coll_overlap:
                if not skip_land:
                    for p in range(4):
                        agp = agp_drams[it][p]
                        for hh in range(2):
                            nc.sync.dma_start(
                                out=table[:, hh * HALF + p * SUB:
                                          hh * HALF + (p + 1) * SUB],
                                in_=bass.AP(
                                    agp.tensor,
                                    agp[:].offset + hh * SUB,
                                    [(16 * 2 * SUB, 8), (2 * SUB, 16),
                                     (1, SUB)]))
            else:
                for c0, L in PIECES_H if not skip_slab else []:
                    for hh in range(2):
                        p0 = ps.tile([16, 512], f32, tag="ps16", bufs=2)
                        nc.tensor.matmul(p0[:, :L], wlin1h[:, hh, :],
                                         hT[:, c0:c0 + L],
                                         start=True, stop=True)
                        stg = wk2.tile([16, 512], f16, tag="slabstg")
                        nc.vector.tensor_copy(out=stg[:, :L], in_=p0[:, :L])
                        nc.sync.dma_start(
                            out=slab_dram[:, hh * HALF + c0:hh * HALF + c0 + L],
                            in_=stg[:, :L])
                if not skip_coll:
                    if fake_collective:
                        for cc_ in range(NC):
                            nc.sync.dma_start(out=ag_dram[cc_],
                                              in_=slab_dram[:])
                    else:
                        nc.gpsimd.collective_compute(
                            "AllGather", OP.bypass,
                            replica_groups=[list(range(NC))],
                            ins=[slab_dram[:].opt()], outs=[ag_dram[:].opt()])
                if not skip_land:
                    # land the gathered [8, 16, NPAD] as one contiguous DMA
                    # into the un-paired table (partition 16b+f <- core b f)
                    nc.sync.dma_start(
                        out=table[:],
                        in_=bass.AP(ag_dram.tensor, ag_dram[:].offset,
                                    [(16 * NPAD, 8), (NPAD, 16), (1, NPAD)]))

            # ---- C: edge chunks (groups of 4, one table gather per group) ----
            ngroups = EPT // 2048

            # scans are software-pipelined one chunk behind: scan(i) enters
            # the in-order DVE queue only after chunk i+1's select/mult, so
            # it never stalls the queue waiting on the PE msg round-trip
            pend_scan = []

            def flush_scan():
                while pend_scan:
                    pend_scan.pop(0)()

            def c_group(gv, lag=False):
                gbuf = gbufs[gv % 2]
                if 'cgather' not in skip:
                    nc.gpsimd.ap_gather(
                        out_ap=gbuf[:].bitcast(f32),
                        in_ap=table[:].bitcast(f32),
                        idxs_ap=dyn(gidx[:, 0:128], gv * 128),
                        channels=128, num_elems=NPAD // 2, d=1, num_idxs=2048)
                for i in range(4):
                    if 'cdma' in skip:
                        cc = ccP
                    else:
                        cc = wk.tile([128, ECH * 7], f16, **BUFA)
                        nc.sync.dma_start(
                            out=cc[:],
                            in_=dyn(cexp_dram[i], gv * (4 * 128 * ECH * 7)))
                    # parity select: xs = par ? odd : even
                    g_even = bass.AP(gbuf.tensor,
                                     gbuf[:].offset + i * ECH * 2,
                                     [gbuf[:].ap[0], (2, ECH)])
                    g_odd = bass.AP(gbuf.tensor,
                                    gbuf[:].offset + i * ECH * 2 + 1,
                                    [gbuf[:].ap[0], (2, ECH)])
                    c_par = bass.AP(cc.tensor, cc[:].offset + 6,
                                    [cc[:].ap[0], (7, ECH)])
                    xs = wk2.tile([128, ECH], f16, tag="xsel")
                    if 'cvec' not in skip:
                        nc.vector.tensor_copy(out=xs[:], in_=g_even)
                        nc.vector.copy_predicated(out=xs[:],
                                                  mask=c_par.bitcast(i16),
                                                  data=g_odd)
                    sc = wk.tile([128, 5, ECH], f16, **BUFB)
                    x_in0 = bass.AP(xs.tensor, xs[:].offset,
                                    [xs[:].ap[0], (0, 5), (1, ECH)])
                    c_in1 = bass.AP(cc.tensor, cc[:].offset,
                                    [cc[:].ap[0], (1, 5), (7, ECH)])
                    if 'cvec' not in skip:
                        nc.vector.tensor_tensor(out=sc[:], in0=x_in0,
                                                in1=c_in1, op=OP.mult)
                    flush_scan()
                    msg = ps.tile([128, 512], f32, tag="ps")
                    if 'cmm' not in skip:
                        for p in range(5):
                            nc.tensor.matmul(msg[:, :ECH], wM_s[:, p, :],
                                             sc[:, p, :],
                                             start=(p == 0), stop=(p == 4))
                    cum_out = dyn(bass.AP(
                        cum.tensor, cum[:].offset + (1 + i * ECH) * 2,
                        [cum[:].ap[0], (2, ECH)]), gv * 4096)
                    init = dyn(bass.AP(
                        cum.tensor, cum[:].offset + i * ECH * 2,
                        [cum[:].ap[0], (1, 1)]), gv * 4096)
                    c_mask = bass.AP(cc.tensor, cc[:].offset + 5,
                                     [cc[:].ap[0], (7, ECH)])
                    if 'cscan' not in skip:
                        pend_scan.append(
                            lambda co=cum_out, cm=c_mask, m=msg, ii=init:
                            nc.vector.tensor_tensor_scan(
                                out=co, data0=cm, data1=m[:, :ECH],
                                initial=ii, op0=OP.mult, op1=OP.add))
                        if not lag:
                            flush_scan()

            # ---- D+E: ends + gates, aligned to sub-chunks ----
            ebs_store = {}

            def d_gather(chv, half):
                eb = ebp.tile([128, SUB, 2], f16,
                              tag=("ebA" if half == 0 else "ebB"))
                if 'dgather' in skip:
                    nc.vector.memset(eb[:], 0)
                else:
                    nc.gpsimd.ap_gather(
                        out_ap=eb[:].bitcast(f32),
                        in_ap=dyn(cum[:, 1:1 + SEG, :].bitcast(f32),
                                  chv * SEG + half * (4 * SEG)),
                        idxs_ap=dyn(
                            eidx[:, 0:SUB // 16],
                            chv * (SUB // 16) + half * (4 * (SUB // 16))),
                        channels=128, num_elems=SEG, d=1, num_idxs=SUB)
                ebs_store[(chv, half)] = eb

            def d_gates(chv):
                whs_cur = CUR['whs']
                wfold_cur = CUR['wfold']
                bias_cur = CUR['bias']
                ebs = [ebs_store.pop((chv, 0)), ebs_store.pop((chv, 1))]
                for off, L in PIECES_S:
                    pr = ps.tile([128, 512], f32, tag="ps")
                    pz = ps.tile([128, 512], f32, tag="ps")
                    pn = ps.tile([128, 512], f32, tag="ps")
                    ph = ps.tile([128, 512], f32, tag="ps")
                    if 'dmm' not in skip:
                        for g, pst in enumerate([pr, pz, pn, ph]):
                            nc.tensor.matmul(pst[:, :L], whs_cur[:, g, :],
                                             dyn(hT[:, off:off + L], chv * SUB),
                                             start=True, stop=True)
                        for g, pst in enumerate([pr, pz, pn]):
                            for half in range(2):
                                eb = ebs[half]
                                rhs = bass.AP(eb.tensor, eb[:].offset + off * 2,
                                              [eb[:].ap[0], (2, L)])
                                out = bass.AP(
                                    pst.tensor,
                                    pst[:].offset + 64 * half * pst[:].ap[0][0],
                                    [(pst[:].ap[0][0], 64), (1, L)])
                                tp = (0, 64) if half == 1 else None
                                nc.tensor.matmul(out, wfold_cur[:, g, :], rhs,
                                                 start=False, stop=False,
                                                 skip_group_check=True,
                                                 tile_position=tp)
                    if 'dvec' in skip:
                        continue
                    r16 = wk2.tile([128, 512], f16, tag="g_r")
                    z16 = wk2.tile([128, 512], f16, tag="g_z")
                    nc.scalar.activation(out=r16[:, :L], in_=pr[:, :L],
                                         func=AF.Sigmoid,
                                         bias=bias_cur[:, 0:1], scale=1.0)
                    nc.scalar.activation(out=z16[:, :L], in_=pz[:, :L],
                                         func=AF.Sigmoid,
                                         bias=bias_cur[:, 1:2],
                                         scale=1.0)
                    t16 = wk2.tile([128, 512], f16, tag="g_t")
                    nc.vector.scalar_tensor_tensor(
                        out=t16[:, :L], in0=ph[:, :L],
                        scalar=bias_cur[:, 3:4], in1=r16[:, :L],
                        op0=OP.add, op1=OP.mult)
                    u16 = wk2.tile([128, 512], f16, tag="g_u")
                    nc.vector.tensor_tensor(out=u16[:, :L], in0=t16[:, :L],
                                            in1=pn[:, :L], op=OP.add)
                    n16 = wk2.tile([128, 512], f16, tag="g_n")
                    nc.scalar.activation(out=n16[:, :L], in_=u16[:, :L],
                                         func=AF.Tanh,
                                         bias=bias_cur[:, 2:3],
                                         scale=1.0)
                    v16 = wk2.tile([128, 512], f16, tag="g_t")
                    nc.vector.tensor_tensor(out=v16[:, :L],
                                            in0=dyn(hT[:, off:off + L],
                                                    chv * SUB),
                                            in1=n16[:, :L], op=OP.subtract)
                    w16 = wk2.tile([128, 512], f16, tag="g_u")
                    nc.vector.tensor_tensor(out=w16[:, :L], in0=z16[:, :L],
                                            in1=v16[:, :L], op=OP.mult)
                    nc.vector.tensor_tensor(out=dyn(hT[:, off:off + L],
                                                    chv * SUB),
                                            in0=n16[:, :L],
                                            in1=w16[:, :L], op=OP.add)

            def d_chunk(chv):
                d_gather(chv, 0)
                d_gather(chv, 1)
                d_gates(chv)

            def after_gates(chv):
                if coll_overlap and it + 1 < niter:
                    if not skip_slab:
                        slab_part(chv)
                    if not skip_coll:
                        ag_part(it + 1, chv)

            interleave = (SEG == 1280 and ngroups == 5 and 'cstage' not in skip
                          and 'dstage' not in skip)
            if interleave:
                # weave the ends-gathers (Pool engine) into the C stream as
                # soon as their cum segments are complete, so they overlap the
                # C-side DVE/PE work instead of serializing after it; gather
                # needs groups 0..ceil((chv+1+4*half)*SEG/2048)-1 done.
                # lag=True defers each scan until the NEXT chunk's DVE prep is
                # queued; every d_gather below still sees its needed scans
                # emitted (the pending scan covers positions none of them
                # read), except after c_group(4) where we flush explicitly.
                c_group(0, lag=True)
                c_group(1, lag=True)
                d_gather(0, 0)          # pos < 1280  <= g0
                c_group(2, lag=True)
                d_gather(1, 0)          # pos < 2560  <= g1
                c_group(3, lag=True)
                d_gather(0, 1)          # pos < 6400  <= g3
                d_gather(1, 1)          # pos < 7680  <= g3 (pending covers 7680+)
                c_group(4, lag=True)
                flush_scan()
                d_gates(0)
                after_gates(0)
                d_gather(2, 0)
                d_gather(2, 1)
                d_gates(1)
                after_gates(1)
                d_gather(3, 0)
                d_gather(3, 1)
                d_gates(2)
                after_gates(2)
                d_gates(3)
                after_gates(3)
                return

            if 'cstage' in skip:
                pass
            elif inner_loops:
                with tc.For_i(0, ngroups) as gv:
                    c_group(gv)
            else:
                for gv in range(ngroups):
                    c_group(gv)
            if 'dstage' in skip:
                pass
            elif inner_loops:
                with tc.For_i(0, 4) as chv:
                    d_chunk(chv)
            else:
                for chv in range(4):
                    d_chunk(chv)
            if coll_overlap:
                for chv in range(4):
                    after_gates(chv)

        # prologue: first iteration's slab + partial AllGathers come from
        # the lin0 output directly
        if coll_overlap and niter > 0:
            emit_slab_ag(0)
        if niter > 0:
            refresh_layer(0, 0)
        for it in range(niter):
            layer = (it // 2) % NL1
            slot = layer % 2
            CUR['whs'] = whs_curs[slot]
            CUR['wfold'] = wfold_curs[slot]
            CUR['bias'] = bias_curs[slot]
            one_iter(it)
            if it % 2 == 0 and it + 2 < niter:
                # prefetch next layer's weights into the other slot while
                # this layer's second iteration runs
                refresh_layer(((it // 2) + 1) % NL1, slot ^ 1)

        # ================= FINAL: y + pooling =================
        for c0, L in PIECES_H:
            py = ps.tile([2, 512], f32, tag="ps")
            nc.tensor.matmul(py[:, :L], wy[:], hT[:, c0:c0 + L],
                             start=True, stop=True)
            ystg = wk2.tile([2, 512], f32, **GGT)
            nc.vector.tensor_copy(out=ystg[:, :L], in_=py[:, :L])
            nc.sync.dma_start(out=y_dram[:, c0:c0 + L], in_=ystg[:, :L])
        ypool = ebp.tile([128, SUB], f32, tag="ebA")
        for half in range(2):
            src = bass.AP(y_dram.tensor, y_dram[:].offset + half * HALF,
                          [(SUB, 4), (0, 16), (1, SUB)])
            nc.sync.dma_start(out=ypool[:][64 * half:64 * (half + 1)], in_=src)
        ycum = ebp.tile([128, SUB], f32, tag="ebB")
        nc.vector.tensor_tensor_scan(out=ycum[:], data0=pmask[:],
                                     data1=ypool[:], initial=0.0,
                                     op0=OP.mult, op1=OP.add)
        pooled = wk2.tile([128, GSLOT], f32, tag="g_r")
        nc.gpsimd.ap_gather(out_ap=pooled[:], in_ap=ycum[:], idxs_ap=pidx[:],
                            channels=128, num_elems=SUB, d=1, num_idxs=GSLOT)
        nc.sync.dma_start(out=out_d[:], in_=pooled[:][0::16])

    nc.compile()
    return nc


# ================= driver =================
_CACHE = {}


def kernel(**inputs):
    inputs = {k: np.asarray(v) for k, v in inputs.items()}
    in_maps_data, meta = host_prep(inputs)
    w = fold_weights_host(inputs)
    EPT = meta["EPT"]
    if EPT not in _CACHE:
        _CACHE[EPT] = build(EPT)
    nc = _CACHE[EPT]
    from concourse.bass_utils import run_bass_kernel_spmd
    in_maps = build_in_maps(in_maps_data, w, EPT)
    trace = os.environ.get("KERNEL_PROFILE", "0") == "1"
    br = run_bass_kernel_spmd(nc, in_maps, list(range(NC)), trace=trace)
    if trace and br.exec_time_ns is not None:
        print(f"HW exec time: {br.exec_time_ns} ns")
    got = np.zeros(NG, np.float32)
    for c in range(NC):
        pooled = br.results[c]["pooled"]
        for s in range(8):
            for i, (g, endpos) in enumerate(meta["pool_graphs"][c][s]):
                got[g] = pooled[s, i]
    return got

